# revision 48
# baseline (speedup 1.0000x reference)
"""Trainium2 Bass kernel for nn_CESAR_24309514895978 (ragged_sequence).

Math (per batch b):
  m0 = (attention_masks==1)&(token_type_ids==0); m1 = (attention_masks==1)&(token_type_ids==1)
  score[i,j] = |emb_n[i] . emb_n[j]|   (L2-normalized embeddings)
  logits[i,j] = (emb@Wq.T+bq)[i] . (emb@Wk.T+bk)[j]
  cs[b] = sum_{valid ij} softmax_flat(logits | pair_mask)[i,j] * score[i,j]

Key optimizations over the dense version:
  * Only i in m0 and j in m1 matter (the pair mask kills everything else).
    The host gathers valid tokens per side; the two batches of a core are
    packed CONTIGUOUSLY on one merged axis (batch 1 starts at the runtime
    boundary n_b0), padded to a compile-time ntot (~272 vs dense 512).
    Rank-4 mask rows handle cross-batch/pad exclusion.
  * All matmul inputs bf16: PE runs 1 cycle/row at any free size, LDWEIGHTS
    halves vs fp32r, DMA bytes halve.  rel-err stays ~1e-2 < 2e-2 because
    the flattened softmax is peaked and bf16 logit noise largely cancels
    between numerator and denominator.
  * The gram shares the RAW e0/e1 tiles with the logits path (no separate
    normalized copy -> 0.5MB less HBM traffic); the 1/||e1_j|| scale ships
    as one bf16 row, gpsimd-broadcast to W2 and folded into E on the DVE;
    1/||e0_i|| is applied on the host.  No on-chip sqrt/recip chain, so
    Identity/Exp/Abs/Copy all live in one act table set (exp_and_others).
  * Per-PARTITION (row) softmax max, shipped to host with Z/W partials: no
    cross-chunk all-reduce on device, exp fires right after each L chunk,
    and the host rescales with exp(M_row - M_batch) in fp64.
  * Constant folding: logits = e0aug @ A_aug @ e1aug.T,
    A_aug = [[Wq.T@Wk, Wq.T@bk], [bq.T@Wk, bq.bk]]; u-column rides the
    PSUM->SBUF copy bias, prow+masks ride one K=4 matmul.
  * PE warm-up matmuls on a zeroed tile during the DMA lead-in (TRN2 p-state
    ramps to 2.4GHz only after ~3us of continuous PE activity); a dummy ACT
    op hoists the 1.3us lazy ACT_TABLE_LOAD into the same window.
  * Stage-2 is da-outer so each P chunk is consumed right as its PSUM->SBUF
    copy lands; stage-1 is db-outer to match the at-chunk DMA arrival order.
  * Each issuing engine (sync/scalar/gpsimd) owns one ~110GB/s DMA queue
    (together they saturate the ~350GB/s HBM bus), so every at chunk is cut
    into 3 da-aligned pieces rotated across the queues and each tile has
    exactly one DMA writer (deps are tile-granular).
"""
import numpy as np
import ml_dtypes

import concourse.tile as tile
from concourse import bacc, mybir
from concourse.bass_utils import run_bass_kernel_spmd

B, S, D = 16, 512, 1024
NCORES = 8
BPC = B // NCORES          # batches per core
NCH = D // 128             # 8 contraction chunks
DA = D + 1                 # augmented dim
NEG = np.float32(-1e30)

F32 = mybir.dt.float32
BF16 = mybir.dt.bfloat16
AFT = mybir.ActivationFunctionType
ALU = mybir.AluOpType
AX = mybir.AxisListType

PROFILE = False            # set True (e.g. from test.py) to capture NTFF profile
LAST_RESULTS = None        # BassKernelResults of the last run (for test.py)

_built = {}


def _ic_slices(ntot):
    return [(lo, min(lo + 128, ntot)) for lo in range(0, ntot, 128)]


def _build(ntot, ntj):
    key = (ntot, ntj)
    if key in _built:
        return _built[key]

    ics = _ic_slices(ntot)
    nic = len(ics)

    nc = bacc.Bacc("TRN2", target_bir_lowering=False, debug=False)

    at_d = nc.dram_tensor("at", [128, NCH * DA], BF16, kind="ExternalInput").ap()
    e1t_d = nc.dram_tensor("e1t", [128, NCH * ntj], BF16, kind="ExternalInput").ap()
    e0t_d = nc.dram_tensor("e0t", [128, NCH * ntot], BF16, kind="ExternalInput").ap()
    # rows: prow, R1, R2, NEGrow, ones, A1, A2, Apad, rj
    mask_d = nc.dram_tensor("mask", [9, ntot], BF16, kind="ExternalInput").ap()
    uc_d = nc.dram_tensor("uc", [128, 9], F32, kind="ExternalInput").ap()

    # cols [0:nic]=Z partials, [nic:2nic]=W partials, [2nic:3nic]=-rowmax
    zw_d = nc.dram_tensor("zw", [128, 3 * nic], F32, kind="ExternalOutput").ap()

    with tile.TileContext(nc) as tc:
        with (
            tc.tile_pool(name="atp", bufs=1) as atp,
            tc.tile_pool(name="e1p", bufs=1) as e1p,
            tc.tile_pool(name="e0p", bufs=1) as e0p,
            tc.tile_pool(name="paugp", bufs=NCH) as paugp,
            tc.tile_pool(name="smallp", bufs=1) as smallp,
            tc.tile_pool(name="warmp", bufs=1) as warmp,
            tc.tile_pool(name="Ep", bufs=3) as Ep,
            tc.tile_pool(name="gap", bufs=2) as gap,
            tc.tile_pool(name="scrp", bufs=2) as scrp,
            tc.tile_pool(name="ps", bufs=8, space="PSUM") as ps,
        ):
            # ONE TILE PER DMA (deps are tile-granular).  Each issuing
            # engine (sync/scalar/gpsimd) owns one ~110GB/s DMA queue and
            # the three together saturate the ~350GB/s HBM bus, so every
            # at chunk is split into 3 da-aligned pieces rotated across the
            # queues: all three queues cooperate on each chunk, arriving at
            # stage-1's per-chunk consumption cadence.
            PIECES = [(0, 384), (384, 768), (768, DA)]  # da-aligned thirds
            at_p = [[atp.tile([128, hi - lo], BF16, tag=f"at{db}_{p}",
                              name=f"at{db}_{p}")
                     for p, (lo, hi) in enumerate(PIECES)]
                    for db in range(NCH)]
            e1pr = [e1p.tile([128, 2 * ntj], BF16, tag=f"e1pr{k}",
                             name=f"e1pr{k}") for k in range(NCH // 2)]
            # e0 pieces by da-chunk ranges {0-2}, {3-5}, {6-7}
            E0R = [(0, 3), (3, 6), (6, 8)]
            e0_p = [e0p.tile([128, (h - l) * ntot], BF16, tag=f"e0_{p}",
                             name=f"e0_{p}") for p, (l, h) in enumerate(E0R)]

            # ---- PE warm-up: DVE-zeroed tile (DVE is idle at start and
            # needs no act table), no DMA deps -> PE busy right after the
            # preamble so the p-state is ramped before real data lands.
            warm = warmp.tile([128, 512], BF16, tag="warm")
            nc.vector.memset(warm[:], 0.0)
            # dummy ACT op: hoists the lazy ACT_TABLE_LOAD (1.3us) into the
            # DMA lead-in so it cannot delay the prow copy later (writes its
            # own scratch tile -- must NOT write warm, or warm-up waits on it)
            actscr = warmp.tile([1, 2], F32, tag="actscr")
            nc.scalar.copy(out=actscr[:], in_=warm[0:1, 0:2])
            warm_ps = ps.tile([128, 512], F32, tag="ps", name="warm_ps")
            # distinct widths: identical matmuls can get deduplicated
            for w in (512, 511, 510, 509, 448, 384):
                nc.tensor.matmul(warm_ps[:, 0:w], warm[:, 0:128],
                                 warm[:, 0:w], start=True, stop=True)

            # ---- DMA issues: round-robin in consumption order.  Queue q
            # gets piece (q-db)%3 of chunk db and every third e1t chunk,
            # then an e0 piece, then the small tensors.
            engs = [nc.sync, nc.scalar, nc.gpsimd]
            rrm_t = smallp.tile([4, ntj], BF16, tag="rrm")
            lrm_t = smallp.tile([4, ntot], BF16, tag="lrm")
            rj_t = smallp.tile([1, ntj], BF16, tag="rj")
            uc_t = smallp.tile([128, 9], F32, tag="uc")
            at00x = atp.tile([128, 128], BF16, tag="at00x")
            for q, eng in enumerate(engs):
                if q == 2:  # gpsimd: first e1t pair before its at pieces
                    eng.dma_start(out=e1pr[0][:], in_=e1t_d[:, 0 : 2 * ntj])
                if q == 0:  # sync: tiny first slice of chunk0 piece0
                    eng.dma_start(out=at00x[:], in_=at_d[:, 0:128])
                for db in range(NCH):
                    p = (q - db) % 3
                    lo, hi = PIECES[p]
                    if db == 0 and p == 0:
                        lo = 128  # rest of piece0 (at00x carries [0:128])
                    eng.dma_start(out=at_p[db][p][:, lo - PIECES[p][0]:],
                                  in_=at_d[:, db * DA + lo : db * DA + hi])
                    if db in (3, 5, 7) and db % 3 == q - 0 and False:
                        pass
                    if db in (2, 4, 6) and q == (db // 2 - 1) % 3:
                        k = db // 2
                        eng.dma_start(out=e1pr[k][:],
                                      in_=e1t_d[:, db * ntj : (db + 2) * ntj])
                l, h = E0R[q]
                eng.dma_start(out=e0_p[q][:],
                              in_=e0t_d[:, l * ntot : h * ntot])
            nc.sync.dma_start(out=uc_t[:], in_=uc_d)
            nc.scalar.dma_start(out=rrm_t[:], in_=mask_d[0:4, 0:ntj])
            nc.gpsimd.dma_start(out=lrm_t[:], in_=mask_d[4:8, :])
            nc.gpsimd.dma_start(out=rj_t[:], in_=mask_d[8:9, 0:ntj])
            # W2 = r_j broadcast over partitions (for |G| * r_j)
            W2 = smallp.tile([128, ntj], BF16, tag="W2")
            nc.gpsimd.partition_broadcast(W2[:], rj_t[0:1, :], channels=128)

            def atsl(db, lo, hi):
                if db == 0 and hi <= 128:
                    return at00x[:, lo:hi]
                p = 0 if hi <= 384 else (1 if hi <= 768 else 2)
                base = PIECES[p][0]
                return at_p[db][p][:, lo - base : hi - base]

            def e1sl(db):
                return e1pr[db // 2][:, (db % 2) * ntj : (db % 2 + 1) * ntj]

            def e0sl(c, lo, hi):
                p = 0 if c < 3 else (1 if c < 6 else 2)
                base = E0R[p][0]
                return e0_p[p][:, (c - base) * ntot + lo : (c - base) * ntot + hi]

            # ---- stage 1: P = A_aug @ e1augT, db-outer (DMA arrival
            # order), all 8 da banks in one pass (prow is host-computed)
            P_ps = [
                ps.tile([128, ntj], F32, tag="ps", name=f"P{da}")
                for da in range(NCH)
            ]
            for db in range(NCH):
                st = db == 0
                sp = db == NCH - 1
                for da in range(NCH):
                    nc.tensor.matmul(
                        P_ps[da][:], atsl(db, da * 128, (da + 1) * 128), e1sl(db),
                        start=st, stop=sp,
                    )

            # ---- PSUM->SBUF copies with the u-column bias, ACT/DVE split
            paug = []
            for da in range(NCH):
                pt = paugp.tile([128, ntj], BF16, tag="paug", name=f"paug{da}")
                if da % 2 == 0:
                    nc.scalar.activation(out=pt[:], in_=P_ps[da][:],
                                         func=AFT.Identity,
                                         bias=uc_t[:, da : da + 1], scale=1.0)
                else:
                    nc.vector.tensor_scalar_add(pt[:], P_ps[da][:],
                                                uc_t[:, da : da + 1])
                paug.append(pt)

            # ---- stage 2: L chunks, da-outer (consumes paug as produced),
            # then the rank-4 mask/prow matmul and per-ROW max per chunk.
            L_ps = [
                ps.tile([128, ntj], F32, tag="ps", name=f"L{ic}")
                for ic in range(nic)
            ]
            zw_t = smallp.tile([128, 3 * nic], F32, tag="zw")
            for da in range(NCH):
                for ic, (lo, hi) in enumerate(ics):
                    m = hi - lo
                    nc.tensor.matmul(L_ps[ic][0:m, :], e0sl(da, lo, hi),
                                     paug[da][:], start=(da == 0), stop=False)
            for ic, (lo, hi) in enumerate(ics):
                m = hi - lo
                nc.tensor.matmul(L_ps[ic][0:m, :], lrm_t[:, lo:hi],
                                 rrm_t[:], start=False, stop=True)
                nc.vector.reduce_max(zw_t[0:m, 2 * nic + ic : 2 * nic + ic + 1],
                                     L_ps[ic][0:m, :], axis=AX.X, negate=True)

            # ---- gram chunks (j-side pre-normalized on host), ic-outer so
            # each G finishes early for the abs/stt pipeline
            G_ps = []
            for ic, (lo, hi) in enumerate(ics):
                m = hi - lo
                Gp = ps.tile([128, ntj], F32, tag="ps", name=f"G{ic}")
                for c in range(NCH):
                    nc.tensor.matmul(Gp[0:m, :], e0sl(c, lo, hi), e1sl(c),
                                     start=(c == 0), stop=(c == NCH - 1))
                G_ps.append(Gp)

            # ---- E = exp(L - rowmax) + Z row-accum on ACT (fires during
            # gram); W = sum (|G| * E) in ONE fused DVE stt per chunk:
            # (G abs_max 0) mult E, with row accumulation
            E_t = []
            for ic, (lo, hi) in enumerate(ics):
                m = hi - lo
                E = Ep.tile([128, ntj], BF16, tag="E", name=f"E{ic}")
                nc.scalar.activation(out=E[0:m, :], in_=L_ps[ic][0:m, :],
                                     func=AFT.Exp,
                                     bias=zw_t[0:m, 2 * nic + ic : 2 * nic + ic + 1],
                                     scale=1.0,
                                     accum_out=zw_t[0:m, ic : ic + 1])
                Ew = Ep.tile([128, ntj], BF16, tag="Ew", name=f"Ew{ic}")
                nc.vector.tensor_mul(Ew[0:m, :], E[0:m, :], W2[0:m, :])
                E_t.append(Ew)
            for ic, (lo, hi) in enumerate(ics):
                m = hi - lo
                ga = gap.tile([128, ntj], BF16, tag="ga", name=f"ga{ic}")
                nc.scalar.activation(out=ga[0:m, :], in_=G_ps[ic][0:m, :],
                                     func=AFT.Abs, bias=0.0, scale=1.0)
                scr = scrp.tile([128, ntj], BF16, tag="scr", name=f"scr{ic}")
                nc.vector.scalar_tensor_tensor(
                    out=scr[0:m, :], in0=ga[0:m, :], scalar=1.0,
                    in1=E_t[ic][0:m, :], op0=ALU.mult, op1=ALU.mult,
                    accum_out=zw_t[0:m, nic + ic : nic + ic + 1])

            nc.sync.dma_start(out=zw_d, in_=zw_t[:])

    nc.compile()
    _built[key] = nc
    return nc


def kernel(embeddings, Wq, bq, Wk, bk, attention_masks, token_type_ids):
    global LAST_RESULTS

    emb = np.ascontiguousarray(np.asarray(embeddings, dtype=np.float32))
    Wq = np.asarray(Wq, dtype=np.float32)
    Wk = np.asarray(Wk, dtype=np.float32)
    bq = np.asarray(bq, dtype=np.float32)
    bk = np.asarray(bk, dtype=np.float32)
    am = np.asarray(attention_masks)
    tt = np.asarray(token_type_ids)

    tok = am == 1
    m0 = tok & (tt == 0)
    m1 = tok & (tt == 1)
    n0 = m0.sum(1)
    n1 = m1.sum(1)

    # merged-axis width: max per-core pair sum, rounded up to 16
    pair0 = n0.reshape(NCORES, BPC).sum(1)
    pair1 = n1.reshape(NCORES, BPC).sum(1)
    ntot = int(-(-int(pair0.max()) // 16)) * 16      # i-axis width
    ntj = int(-(-int(pair1.max()) // 4)) * 4         # j-axis (free-dim) width
    ics = _ic_slices(ntot)
    nic = len(ics)
    nc = _build(ntot, ntj)

    # ---- constant folding (host, fp64 for accuracy)
    Wq64, Wk64 = Wq.astype(np.float64), Wk.astype(np.float64)
    A_aug = np.empty((DA, DA), np.float64)
    A_aug[:D, :D] = Wq64.T @ Wk64
    A_aug[:D, D] = Wq64.T @ bk.astype(np.float64)    # u
    A_aug[D, :D] = Wk64.T @ bq.astype(np.float64)    # v
    A_aug[D, D] = float(bq.astype(np.float64) @ bk.astype(np.float64))
    # at[p, db*DA + da] = A_aug[da, db*128+p]
    at = np.ascontiguousarray(
        A_aug.T[:D].astype(np.float32).reshape(NCH, 128, DA).transpose(1, 0, 2)
    ).astype(ml_dtypes.bfloat16).reshape(128, NCH * DA)

    uc = np.zeros((128, 9), np.float32)
    uc[:, :NCH] = A_aug[:D, D].astype(np.float32).reshape(NCH, 128).T
    uc[0, 8] = A_aug[D, D]

    def to_chunks(x2):  # [w, D] -> [128, NCH*w] bf16
        w = x2.shape[0]
        return np.ascontiguousarray(
            x2.T.reshape(NCH, 128, w).transpose(1, 0, 2)
        ).astype(ml_dtypes.bfloat16).reshape(128, NCH * w)

    in_maps = []
    r0g = []     # per core: r_i of the merged i-axis rows
    for i in range(NCORES):
        b0, b1 = BPC * i, BPC * i + 1
        e0all = np.zeros((ntot, D), np.float32)
        e1all = np.zeros((ntj, D), np.float32)
        g00, g01 = emb[b0, m0[b0]], emb[b1, m0[b1]]
        g10, g11 = emb[b0, m1[b0]], emb[b1, m1[b1]]
        c0i, c1i = n0[b0], n1[b0]
        e0all[:c0i] = g00
        e0all[c0i : c0i + n0[b1]] = g01
        e1all[:c1i] = g10
        e1all[c1i : c1i + n1[b1]] = g11
        nr0 = np.linalg.norm(
            e0all[: c0i + n0[b1]].astype(np.float64), axis=1)
        r0g.append(1.0 / np.maximum(nr0, 1e-12))
        nr1 = np.linalg.norm(
            e1all[: c1i + n1[b1]].astype(np.float64), axis=1)
        rj = np.zeros(ntj, np.float64)
        rj[: c1i + n1[b1]] = 1.0 / np.maximum(nr1, 1e-12)

        # mask rows [R1, R2, NEGrow, ones, A1, A2, Apad, rj]; on device row 0
        # becomes prow, giving rhs=[prow,R1,R2,NEGrow], lhsT=[ones,A1,A2,Apad]
        mw = max(ntot, ntj)
        mask = np.zeros((9, mw), np.float32)
        nreal1 = c1i + n1[b1]
        mask[0, :nreal1] = (
            e1all[:nreal1].astype(np.float64) @ A_aug[D, :D] + A_aug[D, D]
        ).astype(np.float32)                     # prow = v.e1 + c0
        mask[1:4, :ntj] = NEG
        mask[1, :c1i] = 0.0                      # R1
        mask[2, c1i : c1i + n1[b1]] = 0.0        # R2
        mask[4] = 1.0                            # ones
        mask[5, :c0i] = 1.0                      # A1
        mask[6, c0i : c0i + n0[b1]] = 1.0        # A2
        mask[7] = 1.0 - mask[5] - mask[6]        # Apad
        mask[8, :ntj] = rj                       # gram column scale

        in_maps.append({
            "at": at,
            "e1t": to_chunks(e1all),
            "e0t": to_chunks(e0all),
            "mask": mask.astype(ml_dtypes.bfloat16),
            "uc": uc,
        })

    res = run_bass_kernel_spmd(nc, in_maps, core_ids=list(range(NCORES)),
                               trace=PROFILE)
    LAST_RESULTS = res

    # ---- host reduction: per-row partials -> per-batch softmax-weighted sum
    valid = m0.any(axis=1) & m1.any(axis=1)
    cs = np.zeros(B, np.float64)
    for i in range(NCORES):
        zw = res.results[i]["zw"].astype(np.float64)  # [128, 3*nic]
        b0 = BPC * i
        starts = [0, n0[b0]]
        for s in range(BPC):
            b = b0 + s
            if not valid[b]:
                continue
            g = starts[s] + np.arange(n0[b])      # merged-axis rows
            ic_idx = g // 128
            p_idx = g % 128
            zrow = zw[p_idx, ic_idx]
            wrow = zw[p_idx, nic + ic_idx]
            mrow = -zw[p_idx, 2 * nic + ic_idx]   # per-row max M_i
            mb = mrow.max()
            scale = np.exp(mrow - mb)
            z = (zrow * scale).sum()
            w = (wrow * scale * r0g[i][g]).sum()
            cs[b] = w / (z + 1e-300)
    return cs.astype(np.float32)
